# revision 3
# baseline (speedup 1.0000x reference)
"""Trainium2 Bass kernel for nn_MemoryUnit (scatter_memory).

Computes: att = softmax(x @ mem.T / 0.5); att = hard_shrink_relu(att, 0.005);
att = att / max(L1(att), eps); out = att @ mem.

Sharding: data-parallel over N across 8 cores; mem replicated per core.

Per 128-row tile (m = 2048 memory slots), with the softmax denominator
cancelled by the L1 renormalization:
  logits = x16 @ mem16.T          (single fp16 matmul, fp32 PSUM accum)
  e = exp(2*logits) (f16), s1 = rowsum(e)
  t = lam * s1
  g = e * (e > t) (f16), S = rowsum(g)
  out = (g @ mem16) / max(S, tiny)

Engine placement (the point of this version):
  - PE only does the two real matmuls + one small out-transpose.
  - All big transposes (x tile, g tile) go through the DMA XBAR
    (dma_start_transpose), which runs on the DMA engines, not PE.
  - exp stays on ACT; the g-mask pass on DVE reads/writes f16 (2x rate).
  - threshold/max bookkeeping on GPSIMD; f32->f16 loads via GPSIMD SWDGE.
"""

import sys

sys.path.insert(0, "/opt/trn_rl_repo")

import numpy as np

N_FULL = 131072
Z = 128
M = 2048
P = 128
N_CORES = 8
LAM = 0.005

_cache = {}


def _build(n_rows: int):
    import concourse.bass as bass
    import concourse.bacc as bacc
    import concourse.mybir as mybir
    import concourse.tile as tile
    from concourse.masks import make_identity

    f32 = mybir.dt.float32
    f16 = mybir.dt.float16
    Alu = mybir.AluOpType
    Act = mybir.ActivationFunctionType

    NT = n_rows // P
    assert n_rows % P == 0
    NC_CH = M // P      # 16 mem chunks
    HB = M // 2         # 1024: exp half width

    nc = bacc.Bacc("TRN2", target_bir_lowering=False, debug=False, num_devices=1)
    x_d = nc.dram_tensor("x", [n_rows, Z], f32, kind="ExternalInput")
    mem_d = nc.dram_tensor("mem", [M, Z], f32, kind="ExternalInput")
    out_d = nc.dram_tensor("out", [n_rows, Z], f32, kind="ExternalOutput")

    with tile.TileContext(nc) as tc:
        with (
            tc.tile_pool(name="consts", bufs=1) as consts,
            tc.tile_pool(name="xp", bufs=3) as xp,
            tc.tile_pool(name="xtp", bufs=3) as xtp,
            tc.tile_pool(name="ep", bufs=4) as ep,
            tc.tile_pool(name="gp", bufs=3) as gp,
            tc.tile_pool(name="gtp", bufs=3) as gtp,
            tc.tile_pool(name="scal", bufs=6) as scal,
            tc.tile_pool(name="outp", bufs=3) as outp,
            tc.tile_pool(name="lps", bufs=2, space="PSUM") as lps,
            tc.tile_pool(name="ops", bufs=2, space="PSUM") as ops,
            tc.tile_pool(name="tps", bufs=2, space="PSUM") as tps,
        ):
            # ---------- preamble ----------
            identf = consts.tile([P, P], f32)
            make_identity(nc, identf[:])
            ident16 = consts.tile([P, P], f16)
            nc.vector.tensor_copy(out=ident16[:], in_=identf[:])

            # mem as f16 chunks: mh[p, c, z] = mem[c*128+p, z]  (mm2 lhsT)
            mh = consts.tile([P, NC_CH, Z], f16)
            nc.gpsimd.dma_start(
                mh[:], mem_d.ap().rearrange("(c p) z -> p c z", p=P)
            )
            # mem^T via DMA XBAR: mhT[z, c, p] = mem[c*128+p, z] -> flat [z, m]
            mhT = consts.tile([P, NC_CH, P], f16)
            nc.sync.dma_start_transpose(mhT[:], mh[:])

            # ---------- pipeline state ----------
            st = [dict() for _ in range(NT)]

            def stage_dma(i):
                # f32 HBM -> f16 SBUF cast load (GPSIMD software DGE)
                r0 = i * P
                s = st[i]
                s["xh"] = xp.tile([P, Z], f16, tag="xh", name="xh")
                nc.gpsimd.dma_start(s["xh"][:], x_d.ap()[r0:r0 + P, :])

            def stage_xt(i):
                s = st[i]
                s["xhT"] = xtp.tile([P, P], f16, tag="xhT", name="xhT")
                nc.sync.dma_start_transpose(s["xhT"][:], s["xh"][:])

            def stage_mm1(i):
                s = st[i]
                s["logits"] = []
                for h in range(2):
                    logits = lps.tile([P, HB], f32, tag="logits", name="logits")
                    for b in range(2):
                        ii = nc.tensor.matmul(
                            logits[:, b * 512:(b + 1) * 512],
                            s["xhT"][:],
                            mhT[:].rearrange("z c p -> z (c p)")[
                                :, h * HB + b * 512: h * HB + (b + 1) * 512
                            ],
                            start=True, stop=True,
                        )
                        if h > 0 or b > 0:
                            ii.ins.ldweights = False
                    s["logits"].append(logits)
                s.pop("xhT")

            def stage_exp(i):
                s = st[i]
                s["e"] = ep.tile([P, M], f16, tag="e", name="e")
                s["s1h"] = scal.tile([P, 2], f32, tag="s1h", name="s1h")
                # h0 rowsum via ACT accumulator; h1 via DVE reduce (balance)
                nc.scalar.activation(
                    s["e"][:, 0:HB], s["logits"][0][:],
                    Act.Exp, scale=2.0, accum_out=s["s1h"][:, 0:1],
                )
                nc.scalar.activation(
                    s["e"][:, HB:M], s["logits"][1][:],
                    Act.Exp, scale=2.0,
                )
                nc.vector.tensor_reduce(
                    s["s1h"][:, 1:2], s["e"][:, HB:M],
                    axis=mybir.AxisListType.X, op=Alu.add,
                )
                s.pop("logits")

            def stage_t(i):
                s = st[i]
                s1 = scal.tile([P, 1], f32, tag="s1", name="s1")
                nc.gpsimd.tensor_tensor(
                    out=s1[:], in0=s["s1h"][:, 0:1], in1=s["s1h"][:, 1:2],
                    op=Alu.add,
                )
                s["t"] = scal.tile([P, 1], f32, tag="t", name="t")
                nc.gpsimd.tensor_scalar_mul(s["t"][:], s1[:], LAM)
                s.pop("s1h")

            def stage_stt(i):
                s = st[i]
                s["g"] = gp.tile([P, M], f16, tag="g", name="g")
                S = scal.tile([P, 1], f32, tag="S", name="S")
                nc.vector.scalar_tensor_tensor(
                    out=s["g"][:], in0=s["e"][:], scalar=s["t"][:], in1=s["e"][:],
                    op0=Alu.is_gt, op1=Alu.mult, accum_out=S[:],
                )
                Sc = scal.tile([P, 1], f32, tag="Sc", name="Sc")
                nc.gpsimd.tensor_scalar_max(Sc[:], S[:], 1e-30)
                s["rS"] = scal.tile([P, 1], f32, tag="rS", name="rS")
                nc.vector.reciprocal(s["rS"][:], Sc[:])
                s.pop("e")
                s.pop("t")

            def stage_gt(i):
                # g [n, m] -> gT[p, c, n] = g[n, c*128+p] via DMA XBAR
                s = st[i]
                s["gT"] = gtp.tile([P, NC_CH, P], f16, tag="gT", name="gT")
                nc.sync.dma_start_transpose(s["gT"][:], s["g"][:])
                s.pop("g")

            def stage_mm2(i):
                s = st[i]
                outT = ops.tile([P, 512], f32, tag="outT", name="outT")
                s["outT"] = outT
                for c in range(NC_CH):
                    nc.tensor.matmul(
                        outT[:, 0:P], mh[:, c, :], s["gT"][:, c, :],
                        start=(c == 0), stop=(c == NC_CH - 1),
                    )
                s.pop("gT")

            def stage_outd(i):
                s = st[i]
                s["outd"] = outp.tile([P, P], f16, tag="outd", name="outd")
                nc.vector.tensor_copy(out=s["outd"][:], in_=s["outT"][:, 0:P])
                s.pop("outT")

            def stage_outt(i):
                s = st[i]
                bt = tps.tile([P, 1024], f16, tag="bt", name="bt")
                s["bt"] = bt
                nc.tensor.transpose(bt[:, 0:P], s["outd"][:], ident16[:])
                s.pop("outd")

            def stage_fin(i):
                s = st[i]
                fin = outp.tile([P, P], f32, tag="fin", name="fin")
                nc.vector.tensor_scalar_mul(fin[:], s["bt"][:, 0:P], s["rS"][:])
                r0 = i * P
                nc.sync.dma_start(out_d.ap()[r0:r0 + P, :], fin[:])
                s.pop("bt")
                s.pop("rS")

            # ---------- software-pipelined emission ----------
            # Emission order within a step puts PE work that is ready first
            # (out transpose, mm2) ahead of mm1 (which waits on exp freeing
            # its PSUM slot), and lets ACT lead with exp.
            SK_DMA, SK_XT, SK_MM1, SK_EXP, SK_T = 0, 1, 3, 4, 5
            SK_STT, SK_GT, SK_MM2, SK_OUTD, SK_OUTT, SK_FIN = 6, 7, 8, 9, 10, 11
            LAST = SK_FIN
            stages = [
                (SK_DMA, stage_dma),
                (SK_XT, stage_xt),
                (SK_OUTT, stage_outt),
                (SK_MM2, stage_mm2),
                (SK_MM1, stage_mm1),
                (SK_EXP, stage_exp),
                (SK_OUTD, stage_outd),
                (SK_T, stage_t),
                (SK_STT, stage_stt),
                (SK_GT, stage_gt),
                (SK_FIN, stage_fin),
            ]
            for s_idx in range(NT + LAST):
                for skew, fn in stages:
                    i = s_idx - skew
                    if 0 <= i < NT:
                        fn(i)

    nc.compile()
    return nc


def _get_nc(n_rows: int):
    if n_rows not in _cache:
        _cache[n_rows] = _build(n_rows)
    return _cache[n_rows]


def kernel(x: np.ndarray, mem: np.ndarray) -> np.ndarray:
    from concourse.bass_utils import run_bass_kernel_spmd

    x = np.ascontiguousarray(np.asarray(x, dtype=np.float32))
    mem = np.ascontiguousarray(np.asarray(mem, dtype=np.float32))
    n = x.shape[0]
    assert n % N_CORES == 0
    n_loc = n // N_CORES
    nc = _get_nc(n_loc)
    in_maps = [
        {"x": x[i * n_loc:(i + 1) * n_loc], "mem": mem} for i in range(N_CORES)
    ]
    # transient NRT/device errors happen occasionally; retry a couple times
    last_err = None
    for _ in range(3):
        try:
            res = run_bass_kernel_spmd(nc, in_maps, list(range(N_CORES)))
            break
        except Exception as err:  # noqa: BLE001
            last_err = err
            import time as _time
            _time.sleep(10)
    else:
        raise last_err
    out = np.concatenate([r["out"] for r in res.results], axis=0)
    return out.astype(np.float32)
